# revision 11
# baseline (speedup 1.0000x reference)
"""Trainium2 Bass kernel for nn_MessagePassing (10-step 3x3 per-pixel-weighted stencil).

Algorithm (per core, one batch element):
  reference: nw = w / (sum_taps(w)+eps); 10x: x = sum_{di,dj} nw[di,dj] * shift(x, di, dj)

Device formulation ("B stationary, x moving, out[wo,c]"):
  For each output row r:
      psum[wo, c] = sum_{di in 0..2} B[di, r]^T @ x_plane[r+di-1]
  where B[di, r][ws, wo] holds RAW tap weights wt[3*di+dj, r, wo] at ws = wo+dj-1
  (host-side scatter layout, fp16) and x_plane[q] = state[:, q] is [ws=128, c=64].
  The stationary operand is B (128 cols -> fast weight load), the moving operand
  is the state plane (N=64).  Output lands directly in [wo, c] orientation =
  the state layout, so NO transposes are needed.  The per-pixel normalization
  1/(sum_taps+eps) is folded into the PSUM evacuation: vector tensor_mul by a
  c-broadcast reciprocal for most row-groups, scalar-engine activation (per-
  partition scale) per row for the rest, so both engines share the drain work.

Layouts (per core):
  state: [w=128 partitions, plane*64 + c] fp16, 130 planes (plane p = row p-1;
         planes 0 and 129 are zero halos), ping-pong x2.
  B:     [ws=128 partitions, (h-chunk of 16 rows) x (di 3) x (wo 128)] fp16, 8 tiles.
  recip: [w=128, h=128] fp32 = 1/(sum_taps+eps), computed on device.
  psum:  [128, 1024] fp32 = 16 rows x 64 c (2 banks per tile).
"""

import numpy as np

C, H, W = 64, 128, 128
N_CORES = 8
STEPS = 10
EPS = 1e-5
HCH = 16          # h rows per B chunk tile
NB = H // HCH     # 8
PL = 130          # planes per state tensor (halo + 128 rows + halo)
GR = 16           # output rows per psum tile (2 banks)
NG = H // GR      # 8 groups per step


def build_nc():
    import concourse.mybir as mybir
    from concourse import bacc
    from concourse.tile import TileContext

    f32 = mybir.dt.float32
    f16 = mybir.dt.float16

    nc = bacc.Bacc(trn_type="TRN2", target_bir_lowering=False, debug=False)
    xT = nc.dram_tensor("xT", [W, H * C], f16, kind="ExternalInput").ap()
    braw = nc.dram_tensor("braw", [W, H * 3 * W], f16, kind="ExternalInput").ap()
    wtT = nc.dram_tensor("wtT", [W, 9 * H], f32, kind="ExternalInput").ap()
    yT = nc.dram_tensor("yT", [W, H * C], f32, kind="ExternalOutput").ap()

    with TileContext(nc) as tc:
        with (
            tc.tile_pool(name="per", bufs=1) as per,
            tc.tile_pool(name="ps", bufs=3, space="PSUM") as psp,
            tc.tile_pool(name="ps8", bufs=2, space="PSUM") as psp8,
        ):
            # ---- persistent SBUF ----
            Bt = [per.tile([W, HCH * 3 * W], f16, tag=f"B{k}", name=f"B{k}")
                  for k in range(NB)]
            st = [per.tile([W, PL * C], f16, tag=f"st{s}", name=f"st{s}")
                  for s in range(2)]
            wt_sb = per.tile([W, 9 * H], f32, tag="wt")
            sA = per.tile([W, H], f32, tag="sA")
            sB = per.tile([W, H], f32, tag="sB")
            recip = per.tile([W, H], f32, tag="recip")
            stage = per.tile([W, H * C], f32, tag="stage")  # output f32 staging

            # ---- loads, in consumption-priority order ----
            # sync queue: wtT (gates first evac), then interleaved B/x chunks.
            # scalar queue: trailing B chunks in parallel.
            nc.sync.dma_start(out=wt_sb[:], in_=wtT)

            def ldB(k, eng):
                eng.dma_start(
                    out=Bt[k][:], in_=braw[:, k * HCH * 3 * W:(k + 1) * HCH * 3 * W]
                )

            def ldX(j):
                sl = slice(j * 2048, (j + 1) * 2048)
                nc.sync.dma_start(out=st[0][:, C + j * 2048:C + (j + 1) * 2048],
                                  in_=xT[:, sl])

            # x0 and B0 gate the first unit; interleave the rest across both
            # HWDGE queues (sync + scalar) so late B chunks arrive sooner.
            ldX(0)
            ldB(0, nc.sync)
            ldB(1, nc.scalar)
            ldX(1)
            ldB(2, nc.sync)
            ldB(3, nc.scalar)
            ldX(2)
            ldB(4, nc.sync)
            ldB(5, nc.scalar)
            ldX(3)
            ldB(6, nc.sync)
            ldB(7, nc.scalar)

            # zero halos (both ping-pong buffers, never written again)
            for s in range(2):
                nc.vector.memset(st[s][:, 0:C], 0.0)
                nc.vector.memset(st[s][:, 129 * C:PL * C], 0.0)

            # ---- recip = 1/(sum_t wt + eps), [w, h] f32 ----
            nc.vector.tensor_add(out=sA[:], in0=wt_sb[:, 0:H], in1=wt_sb[:, H:2 * H])
            cur, oth = sA, sB
            for t in range(2, 9):
                nc.vector.tensor_add(
                    out=oth[:], in0=cur[:], in1=wt_sb[:, t * H:(t + 1) * H]
                )
                cur, oth = oth, cur
            nc.vector.tensor_scalar_add(out=oth[:], in0=cur[:], scalar1=float(EPS))
            nc.vector.reciprocal(out=recip[:], in_=oth[:])

            # ---- helper APs ----
            def bmat(r, di):  # stationary [ws=128, wo=128] for (out-row r, di)
                k, rr = divmod(r, HCH)
                off = (rr * 3 + di) * W
                return Bt[k][:, off:off + W]

            def plane(s, p):  # moving [ws=128, c=64], plane p of step-s state
                return st[s % 2][:, p * C:(p + 1) * C]

            # ---- the 10 steps, emitted in wavefront order ----
            # Unit (s, g) depends on evacs (s-1, g-1..g+1) and B-chunk g, so
            # emitting by wave = 2s+g lets early steps' low row-groups run
            # while high B-chunks are still in DMA flight (prefetch hiding),
            # and decouples the per-step PE<->DVE coupling.
            # last step uses 8-row groups for a finer end-of-pipeline drain
            units = [(s, GR * g, GR) for s in range(STEPS - 1) for g in range(NG)]
            units += [(STEPS - 1, 8 * g, 8) for g in range(2 * NG)]
            units.sort(key=lambda t: (2 * t[0] + t[1] / GR, t[0], t[1]))
            for s, r0, nr in units:
                last = s == STEPS - 1
                pool = psp if nr == GR else psp8
                ps = pool.tile([W, nr * C], mybir.dt.float32,
                               tag=f"p{nr}", name=f"p{nr}")
                for j in range(nr):
                    r = r0 + j
                    dis = (1, 2) if r == 0 else ((0, 1) if r == H - 1 else (0, 1, 2))
                    for di in dis:
                        nc.tensor.matmul(
                            out=ps[:, j * C:(j + 1) * C],
                            lhsT=bmat(r, di),
                            rhs=plane(s, r + di),
                            start=(di == dis[0]),
                            stop=(di == dis[-1]),
                            # GR independent 64-col accumulation groups share
                            # the banks; the sim group checker conflates them.
                            skip_group_check=True,
                        )
                rc = recip[:, r0:r0 + nr].unsqueeze(2).broadcast_to([W, nr, C])
                pin = ps[:].rearrange("p (j c) -> p j c", j=nr)
                if last:
                    # final step: evacuate normalized f32 straight to the
                    # output staging and DMA out per group.
                    so = stage[:, r0 * C:(r0 + nr) * C].rearrange(
                        "p (j c) -> p j c", j=nr
                    )
                    nc.vector.tensor_mul(out=so, in0=pin, in1=rc)
                    nc.sync.dma_start(
                        out=yT[:, r0 * C:(r0 + nr) * C],
                        in_=stage[:, r0 * C:(r0 + nr) * C],
                    )
                else:
                    so = st[(s + 1) % 2][
                        :, (r0 + 1) * C:(r0 + nr + 1) * C
                    ].rearrange("p (j c) -> p j c", j=nr)
                    nc.vector.tensor_mul(out=so, in0=pin, in1=rc)

    if not nc.is_finalized():
        nc.finalize()
    return nc


def host_prep(inp_i, wt_i):
    """Per-core host-side pure layout/dtype transforms (no arithmetic)."""
    xT = np.ascontiguousarray(inp_i.transpose(2, 1, 0)).reshape(W, H * C)
    # braw[ws, h, di, wo] = wt_i[3*di+dj, h, wo] with ws = wo+dj-1
    braw = np.zeros((W, H, 3, W), dtype=np.float16)
    wo = np.arange(W)
    for di in range(3):
        for dj in range(3):
            ws = wo + dj - 1
            m = (ws >= 0) & (ws < W)
            braw[ws[m], :, di, wo[m]] = wt_i[3 * di + dj][:, wo[m]].T.astype(
                np.float16
            )
    braw = braw.reshape(W, H * 3 * W)
    # wtT[w, t, h] = wt_i[t, h, w]
    wtT = np.ascontiguousarray(wt_i.transpose(2, 0, 1)).reshape(W, 9 * H)
    return {
        "xT": xT.astype(np.float16),
        "braw": braw,
        "wtT": wtT.astype(np.float32),
    }


def unpack(yT):
    return yT.reshape(W, H, C).transpose(2, 1, 0).astype(np.float32)


LAST_RESULTS = None  # BassKernelResults of the most recent kernel() call


def kernel(**inputs):
    import os
    from concourse.bass_utils import run_bass_kernel_spmd

    global LAST_RESULTS
    inp = np.asarray(inputs["input"], dtype=np.float32)
    wt = np.asarray(inputs["weight"], dtype=np.float32)
    n = inp.shape[0]
    in_maps = [host_prep(inp[i], wt[i]) for i in range(n)]
    nc = build_nc()
    trace = bool(int(os.environ.get("MP_TRACE", "0")))
    res = run_bass_kernel_spmd(
        nc, in_maps, core_ids=list(range(n)), trace=trace
    )
    LAST_RESULTS = res
    out = np.stack([unpack(r["yT"]) for r in res.results])
    return out.astype(np.float32)


if __name__ == "__main__":
    # smoke: build only
    nc = build_nc()
    print("built ok")


# revision 14
# speedup vs baseline: 1.0257x; 1.0257x over previous
"""Trainium2 Bass kernel for nn_MessagePassing (10-step 3x3 per-pixel-weighted stencil).

Algorithm (per core, one batch element):
  reference: nw = w / (sum_taps(w)+eps); 10x: x = sum_{di,dj} nw[di,dj] * shift(x, di, dj)

Device formulation ("B stationary, x moving, out[wo,c]"):
  For each output row r:
      psum[wo, c] = sum_{di in 0..2} B[di, r]^T @ x_plane[r+di-1]
  where B[di, r][ws, wo] holds RAW tap weights wt[3*di+dj, r, wo] at ws = wo+dj-1
  (host-side scatter layout, fp16) and x_plane[q] = state[:, q] is [ws=128, c=64].
  The stationary operand is B (128 cols -> fast weight load), the moving operand
  is the state plane (N=64).  Output lands directly in [wo, c] orientation =
  the state layout, so NO transposes are needed.  The per-pixel normalization
  1/(sum_taps+eps) is folded into the PSUM evacuation: vector tensor_mul by a
  c-broadcast reciprocal for most row-groups, scalar-engine activation (per-
  partition scale) per row for the rest, so both engines share the drain work.

Layouts (per core):
  state: [w=128 partitions, plane*64 + c] fp16, 130 planes (plane p = row p-1;
         planes 0 and 129 are zero halos), ping-pong x2.
  B:     [ws=128 partitions, (h-chunk of 16 rows) x (di 3) x (wo 128)] fp16, 8 tiles.
  recip: [w=128, h=128] fp32 = 1/(sum_taps+eps), computed on device.
  psum:  [128, 1024] fp32 = 16 rows x 64 c (2 banks per tile).
"""

import numpy as np

C, H, W = 64, 128, 128
N_CORES = 8
STEPS = 10
EPS = 1e-5
HCH = 16          # h rows per B chunk tile
NB = H // HCH     # 8
PL = 130          # planes per state tensor (halo + 128 rows + halo)
GR = 16           # output rows per psum tile (2 banks)
NG = H // GR      # 8 groups per step


def build_nc():
    import concourse.mybir as mybir
    from concourse import bacc
    from concourse.tile import TileContext

    f32 = mybir.dt.float32
    f16 = mybir.dt.float16

    nc = bacc.Bacc(trn_type="TRN2", target_bir_lowering=False, debug=False)
    xT = nc.dram_tensor("xT", [W, H * C], f16, kind="ExternalInput").ap()
    braw = nc.dram_tensor("braw", [W, H * 3 * W], f16, kind="ExternalInput").ap()
    wtT = nc.dram_tensor("wtT", [W, 9 * H], f32, kind="ExternalInput").ap()
    yT = nc.dram_tensor("yT", [W, H * C], f32, kind="ExternalOutput").ap()

    with TileContext(nc) as tc:
        with (
            tc.tile_pool(name="per", bufs=1) as per,
            tc.tile_pool(name="ps", bufs=4, space="PSUM") as psp,
        ):
            # ---- persistent SBUF ----
            Bt = [per.tile([W, HCH * 3 * W], f16, tag=f"B{k}", name=f"B{k}")
                  for k in range(NB)]
            st = [per.tile([W, PL * C], f16, tag=f"st{s}", name=f"st{s}")
                  for s in range(2)]
            wt_sb = per.tile([W, 9 * H], f32, tag="wt")
            sA = per.tile([W, H], f32, tag="sA")
            sB = per.tile([W, H], f32, tag="sB")
            recip = per.tile([W, H], f32, tag="recip")
            stage = per.tile([W, H * C], f32, tag="stage")  # output f32 staging

            # ---- loads, in consumption-priority order ----
            # sync queue: wtT (gates first evac), then interleaved B/x chunks.
            # scalar queue: trailing B chunks in parallel.
            nc.sync.dma_start(out=wt_sb[:], in_=wtT)

            def ldB(k, eng):
                eng.dma_start(
                    out=Bt[k][:], in_=braw[:, k * HCH * 3 * W:(k + 1) * HCH * 3 * W]
                )

            def ldX(j):
                sl = slice(j * 2048, (j + 1) * 2048)
                nc.sync.dma_start(out=st[0][:, C + j * 2048:C + (j + 1) * 2048],
                                  in_=xT[:, sl])

            # Single queue, strict priority order: the HBM pipe is saturated
            # (~400 GB/s) whatever the queue count, so the only thing that
            # matters is arrival ORDER of the chunks that gate early units.
            ldX(0)
            ldB(0, nc.sync)
            ldB(1, nc.sync)
            ldX(1)
            ldB(2, nc.sync)
            ldX(2)
            ldB(3, nc.sync)
            ldB(4, nc.sync)
            ldX(3)
            for k in range(5, NB):
                ldB(k, nc.sync)

            # zero halos (both ping-pong buffers, never written again)
            for s in range(2):
                nc.vector.memset(st[s][:, 0:C], 0.0)
                nc.vector.memset(st[s][:, 129 * C:PL * C], 0.0)

            # ---- recip = 1/(sum_t wt + eps), [w, h] f32 ----
            nc.vector.tensor_add(out=sA[:], in0=wt_sb[:, 0:H], in1=wt_sb[:, H:2 * H])
            cur, oth = sA, sB
            for t in range(2, 9):
                nc.vector.tensor_add(
                    out=oth[:], in0=cur[:], in1=wt_sb[:, t * H:(t + 1) * H]
                )
                cur, oth = oth, cur
            nc.vector.tensor_scalar_add(out=oth[:], in0=cur[:], scalar1=float(EPS))
            nc.vector.reciprocal(out=recip[:], in_=oth[:])

            # ---- helper APs ----
            def bmat(r, di):  # stationary [ws=128, wo=128] for (out-row r, di)
                k, rr = divmod(r, HCH)
                off = (rr * 3 + di) * W
                return Bt[k][:, off:off + W]

            def plane(s, p):  # moving [ws=128, c=64], plane p of step-s state
                return st[s % 2][:, p * C:(p + 1) * C]

            # ---- the 10 steps, emitted in wavefront order ----
            # Unit (s, g) depends on evacs (s-1, g-1..g+1) and B-chunk g, so
            # emitting by wave = 2s+g lets early steps' low row-groups run
            # while high B-chunks are still in DMA flight (prefetch hiding),
            # and decouples the per-step PE<->DVE coupling.
            units = [(s, GR * g, GR) for s in range(STEPS) for g in range(NG)]
            units.sort(key=lambda t: (2 * t[0] + t[1] / GR, t[0], t[1]))
            for s, r0, nr in units:
                last = s == STEPS - 1
                ps = psp.tile([W, nr * C], mybir.dt.float32, tag="p", name="p")
                for j in range(nr):
                    r = r0 + j
                    dis = (1, 2) if r == 0 else ((0, 1) if r == H - 1 else (0, 1, 2))
                    for di in dis:
                        nc.tensor.matmul(
                            out=ps[:, j * C:(j + 1) * C],
                            lhsT=bmat(r, di),
                            rhs=plane(s, r + di),
                            start=(di == dis[0]),
                            stop=(di == dis[-1]),
                            # GR independent 64-col accumulation groups share
                            # the banks; the sim group checker conflates them.
                            skip_group_check=True,
                        )
                rc = recip[:, r0:r0 + nr].unsqueeze(2).broadcast_to([W, nr, C])
                pin = ps[:].rearrange("p (j c) -> p j c", j=nr)
                if last:
                    # final step: evacuate normalized f32 straight to the
                    # output staging and DMA out per group.
                    so = stage[:, r0 * C:(r0 + nr) * C].rearrange(
                        "p (j c) -> p j c", j=nr
                    )
                    nc.vector.tensor_mul(out=so, in0=pin, in1=rc)
                    nc.sync.dma_start(
                        out=yT[:, r0 * C:(r0 + nr) * C],
                        in_=stage[:, r0 * C:(r0 + nr) * C],
                    )
                else:
                    so = st[(s + 1) % 2][
                        :, (r0 + 1) * C:(r0 + nr + 1) * C
                    ].rearrange("p (j c) -> p j c", j=nr)
                    nc.vector.tensor_mul(out=so, in0=pin, in1=rc)

    if not nc.is_finalized():
        nc.finalize()
    return nc


def host_prep(inp_i, wt_i):
    """Per-core host-side pure layout/dtype transforms (no arithmetic)."""
    xT = np.ascontiguousarray(inp_i.transpose(2, 1, 0)).reshape(W, H * C)
    # braw[ws, h, di, wo] = wt_i[3*di+dj, h, wo] with ws = wo+dj-1
    braw = np.zeros((W, H, 3, W), dtype=np.float16)
    wo = np.arange(W)
    for di in range(3):
        for dj in range(3):
            ws = wo + dj - 1
            m = (ws >= 0) & (ws < W)
            braw[ws[m], :, di, wo[m]] = wt_i[3 * di + dj][:, wo[m]].T.astype(
                np.float16
            )
    braw = braw.reshape(W, H * 3 * W)
    # wtT[w, t, h] = wt_i[t, h, w]
    wtT = np.ascontiguousarray(wt_i.transpose(2, 0, 1)).reshape(W, 9 * H)
    return {
        "xT": xT.astype(np.float16),
        "braw": braw,
        "wtT": wtT.astype(np.float32),
    }


def unpack(yT):
    return yT.reshape(W, H, C).transpose(2, 1, 0).astype(np.float32)


LAST_RESULTS = None  # BassKernelResults of the most recent kernel() call


def kernel(**inputs):
    import os
    from concourse.bass_utils import run_bass_kernel_spmd

    global LAST_RESULTS
    inp = np.asarray(inputs["input"], dtype=np.float32)
    wt = np.asarray(inputs["weight"], dtype=np.float32)
    n = inp.shape[0]
    in_maps = [host_prep(inp[i], wt[i]) for i in range(n)]
    nc = build_nc()
    trace = bool(int(os.environ.get("MP_TRACE", "0")))
    res = run_bass_kernel_spmd(
        nc, in_maps, core_ids=list(range(n)), trace=trace
    )
    LAST_RESULTS = res
    out = np.stack([unpack(r["yT"]) for r in res.results])
    return out.astype(np.float32)


if __name__ == "__main__":
    # smoke: build only
    nc = build_nc()
    print("built ok")


# revision 15
# speedup vs baseline: 1.0273x; 1.0016x over previous
"""Trainium2 Bass kernel for nn_MessagePassing (10-step 3x3 per-pixel-weighted stencil).

Algorithm (per core, one batch element):
  reference: nw = w / (sum_taps(w)+eps); 10x: x = sum_{di,dj} nw[di,dj] * shift(x, di, dj)

Device formulation ("B stationary, x moving, out[wo,c]"):
  For each output row r:
      psum[wo, c] = sum_{di in 0..2} B[di, r]^T @ x_plane[r+di-1]
  where B[di, r][ws, wo] holds RAW tap weights wt[3*di+dj, r, wo] at ws = wo+dj-1
  (host-side scatter layout, fp16) and x_plane[q] = state[:, q] is [ws=128, c=64].
  The stationary operand is B (128 cols -> fast weight load), the moving operand
  is the state plane (N=64).  Output lands directly in [wo, c] orientation =
  the state layout, so NO transposes are needed.  The per-pixel normalization
  1/(sum_taps+eps) is folded into the PSUM evacuation: vector tensor_mul by a
  c-broadcast reciprocal for most row-groups, scalar-engine activation (per-
  partition scale) per row for the rest, so both engines share the drain work.

Layouts (per core):
  state: [w=128 partitions, plane*64 + c] fp16, 130 planes (plane p = row p-1;
         planes 0 and 129 are zero halos), ping-pong x2.
  B:     [ws=128 partitions, (h-chunk of 16 rows) x (di 3) x (wo 128)] fp16, 8 tiles.
  recip: [w=128, h=128] fp32 = 1/(sum_taps+eps), computed on device.
  psum:  [128, 1024] fp32 = 16 rows x 64 c (2 banks per tile).
"""

import numpy as np

C, H, W = 64, 128, 128
N_CORES = 8
STEPS = 10
EPS = 1e-5
HCH = 16          # h rows per B chunk tile
NB = H // HCH     # 8
PL = 130          # planes per state tensor (halo + 128 rows + halo)
GR = 16           # output rows per psum tile (2 banks)
NG = H // GR      # 8 groups per step


def build_nc():
    import concourse.mybir as mybir
    from concourse import bacc
    from concourse.tile import TileContext

    f32 = mybir.dt.float32
    f16 = mybir.dt.float16

    nc = bacc.Bacc(trn_type="TRN2", target_bir_lowering=False, debug=False)
    xT = nc.dram_tensor("xT", [W, H * C], f16, kind="ExternalInput").ap()
    braw = nc.dram_tensor("braw", [W, H * 3 * W], f16, kind="ExternalInput").ap()
    wtT = nc.dram_tensor("wtT", [W, 9 * H], f32, kind="ExternalInput").ap()
    yT = nc.dram_tensor("yT", [W, H * C], f32, kind="ExternalOutput").ap()

    with TileContext(nc) as tc:
        with (
            tc.tile_pool(name="per", bufs=1) as per,
            tc.tile_pool(name="ps", bufs=4, space="PSUM") as psp,
        ):
            # ---- persistent SBUF ----
            Bt = [per.tile([W, HCH * 3 * W], f16, tag=f"B{k}", name=f"B{k}")
                  for k in range(NB)]
            st = [per.tile([W, PL * C], f16, tag=f"st{s}", name=f"st{s}")
                  for s in range(2)]
            wt_sb = per.tile([W, 9 * H], f32, tag="wt")
            sA = per.tile([W, H], f32, tag="sA")
            sB = per.tile([W, H], f32, tag="sB")
            recip = per.tile([W, H], f32, tag="recip")
            stage = per.tile([W, H * C], f32, tag="stage")  # output f32 staging

            # ---- loads: cooperative two-queue, consumption-priority order ----
            # One HWDGE ring sustains only ~220 GB/s; two (sync + scalar) reach
            # ~400.  Split every chunk in half, one half per queue, so both
            # queues deliver the SAME chunk simultaneously and chunk k
            # completes at the aggregate rate, in priority order.
            HB = HCH * 3 * W // 2   # B half-chunk cols

            def ldB(k, half):
                eng = nc.sync if half == 0 else nc.scalar
                o = k * HCH * 3 * W + half * HB
                eng.dma_start(out=Bt[k][:, half * HB:(half + 1) * HB],
                              in_=braw[:, o:o + HB])

            def ldX(j, half):
                eng = nc.sync if half == 0 else nc.scalar
                o = j * 2048 + half * 1024
                eng.dma_start(out=st[0][:, C + o:C + o + 1024],
                              in_=xT[:, o:o + 1024])

            def ldBoth(k):
                ldB(k, 0)
                ldB(k, 1)

            ldBoth(0)
            ldX(0, 0); ldX(0, 1)
            nc.scalar.dma_start(out=wt_sb[:], in_=wtT)  # gates first evac only
            ldBoth(1)
            ldBoth(2)
            ldX(1, 0); ldX(1, 1)
            ldBoth(3)
            ldX(2, 0); ldX(2, 1)
            ldBoth(4)
            ldX(3, 0); ldX(3, 1)
            ldBoth(5)
            ldBoth(6)
            ldBoth(7)

            # zero halos (both ping-pong buffers, never written again)
            for s in range(2):
                nc.vector.memset(st[s][:, 0:C], 0.0)
                nc.vector.memset(st[s][:, 129 * C:PL * C], 0.0)

            # ---- recip = 1/(sum_t wt + eps), [w, h] f32 ----
            nc.vector.tensor_add(out=sA[:], in0=wt_sb[:, 0:H], in1=wt_sb[:, H:2 * H])
            cur, oth = sA, sB
            for t in range(2, 9):
                nc.vector.tensor_add(
                    out=oth[:], in0=cur[:], in1=wt_sb[:, t * H:(t + 1) * H]
                )
                cur, oth = oth, cur
            nc.vector.tensor_scalar_add(out=oth[:], in0=cur[:], scalar1=float(EPS))
            nc.vector.reciprocal(out=recip[:], in_=oth[:])

            # ---- helper APs ----
            def bmat(r, di):  # stationary [ws=128, wo=128] for (out-row r, di)
                k, rr = divmod(r, HCH)
                off = (rr * 3 + di) * W
                return Bt[k][:, off:off + W]

            def plane(s, p):  # moving [ws=128, c=64], plane p of step-s state
                return st[s % 2][:, p * C:(p + 1) * C]

            # ---- the 10 steps, emitted in wavefront order ----
            # Unit (s, g) depends on evacs (s-1, g-1..g+1) and B-chunk g, so
            # emitting by wave = 2s+g lets early steps' low row-groups run
            # while high B-chunks are still in DMA flight (prefetch hiding),
            # and decouples the per-step PE<->DVE coupling.
            units = [(s, GR * g, GR) for s in range(STEPS) for g in range(NG)]
            units.sort(key=lambda t: (2 * t[0] + t[1] / GR, t[0], t[1]))
            for s, r0, nr in units:
                last = s == STEPS - 1
                ps = psp.tile([W, nr * C], mybir.dt.float32, tag="p", name="p")
                for j in range(nr):
                    r = r0 + j
                    dis = (1, 2) if r == 0 else ((0, 1) if r == H - 1 else (0, 1, 2))
                    for di in dis:
                        nc.tensor.matmul(
                            out=ps[:, j * C:(j + 1) * C],
                            lhsT=bmat(r, di),
                            rhs=plane(s, r + di),
                            start=(di == dis[0]),
                            stop=(di == dis[-1]),
                            # GR independent 64-col accumulation groups share
                            # the banks; the sim group checker conflates them.
                            skip_group_check=True,
                        )
                rc = recip[:, r0:r0 + nr].unsqueeze(2).broadcast_to([W, nr, C])
                pin = ps[:].rearrange("p (j c) -> p j c", j=nr)
                if last:
                    # final step: evacuate normalized f32 straight to the
                    # output staging and DMA out per group.
                    so = stage[:, r0 * C:(r0 + nr) * C].rearrange(
                        "p (j c) -> p j c", j=nr
                    )
                    nc.vector.tensor_mul(out=so, in0=pin, in1=rc)
                    nc.sync.dma_start(
                        out=yT[:, r0 * C:(r0 + nr) * C],
                        in_=stage[:, r0 * C:(r0 + nr) * C],
                    )
                else:
                    so = st[(s + 1) % 2][
                        :, (r0 + 1) * C:(r0 + nr + 1) * C
                    ].rearrange("p (j c) -> p j c", j=nr)
                    nc.vector.tensor_mul(out=so, in0=pin, in1=rc)

    if not nc.is_finalized():
        nc.finalize()
    return nc


def host_prep(inp_i, wt_i):
    """Per-core host-side pure layout/dtype transforms (no arithmetic)."""
    xT = np.ascontiguousarray(inp_i.transpose(2, 1, 0)).reshape(W, H * C)
    # braw[ws, h, di, wo] = wt_i[3*di+dj, h, wo] with ws = wo+dj-1
    braw = np.zeros((W, H, 3, W), dtype=np.float16)
    wo = np.arange(W)
    for di in range(3):
        for dj in range(3):
            ws = wo + dj - 1
            m = (ws >= 0) & (ws < W)
            braw[ws[m], :, di, wo[m]] = wt_i[3 * di + dj][:, wo[m]].T.astype(
                np.float16
            )
    braw = braw.reshape(W, H * 3 * W)
    # wtT[w, t, h] = wt_i[t, h, w]
    wtT = np.ascontiguousarray(wt_i.transpose(2, 0, 1)).reshape(W, 9 * H)
    return {
        "xT": xT.astype(np.float16),
        "braw": braw,
        "wtT": wtT.astype(np.float32),
    }


def unpack(yT):
    return yT.reshape(W, H, C).transpose(2, 1, 0).astype(np.float32)


LAST_RESULTS = None  # BassKernelResults of the most recent kernel() call


def kernel(**inputs):
    import os
    from concourse.bass_utils import run_bass_kernel_spmd

    global LAST_RESULTS
    inp = np.asarray(inputs["input"], dtype=np.float32)
    wt = np.asarray(inputs["weight"], dtype=np.float32)
    n = inp.shape[0]
    in_maps = [host_prep(inp[i], wt[i]) for i in range(n)]
    nc = build_nc()
    trace = bool(int(os.environ.get("MP_TRACE", "0")))
    res = run_bass_kernel_spmd(
        nc, in_maps, core_ids=list(range(n)), trace=trace
    )
    LAST_RESULTS = res
    out = np.stack([unpack(r["yT"]) for r in res.results])
    return out.astype(np.float32)


if __name__ == "__main__":
    # smoke: build only
    nc = build_nc()
    print("built ok")
